# revision 53
# baseline (speedup 1.0000x reference)
"""Adjacency-aware multi-head attention on 8 trn2 NeuronCores.

Math (per b, head k):
  Q = h[b] @ Wq[:, k] + bq[k]           [N, D]
  S[i, j] = (Q_i . K_j) / sqrt(D)
  P[j, i] = exp(S[i, j]) / sum_j exp(S[i, j])      (softmax over keys j)
  out[i, d] = sum_j P[j, i] * A[b, j, i] * V[j, d]

The K bias cancels: it adds g[i] = Q_i . bk to every score of query i,
and softmax over j is invariant to per-i shifts -> bk is dropped.

Sharding: 16 (b, head) pairs over 8 cores, 2 heads of the SAME b per core so
the A[b] stream is shared by both heads.

Device dataflow ([j, i] layout so A needs no transpose).  The steady state is
paced by the ACT engine (exp of all scores); everything else hides under it:
  - PE warm-up: dummy matmuls + first half of the V projection run during the
    input-DMA wait so the HAM clock gate reaches 2.4 GHz before the main loop.
  - Strip mapping r = 2*head + (t%2): j-tile t of head hh computes on PE row
    strip r.  Q^T is written strip-replicated directly by the Q projection
    (host passes Wq with columns [h0|h0|h1|h1]), no SBUF->SBUF copies.
  - K^T packed per strip, K projection runs as 8 matmuls of N=1024.
  - exp on ACT: PSUM -> SBUF bf16, 3 PSUM banks per op.
  - EA = E * A on DVE: one 4D-AP tensor_tensor per j-tile pair (2x bf16 mode).
  - phase 2 (eager, lagged ~2 exp-groups): per j-tile 4 streams into one PSUM
    tile via column tiling: out_h0 (cols 0-31), out_h1 (32-63), denom_h0
    (col 64), denom_h1 (col 96).
Device returns [128, N]: rows 0-31 outT_h0, 32-63 outT_h1, rows 64/96 the
softmax denominators.  Host does out = (outT / denom)^T plus the gather.
"""

import math
import os

import numpy as np
import ml_dtypes

B, N, IN_DIM = 2, 2048, 256
HEADS, D = 8, 32
NCORES = 8
HPC = 2              # heads per core
NJ = N // 128        # 16 j-tiles
NCH = 4              # i-chunks
CH = N // NCH        # 512
CORES_PER_B = NCORES // B
GRP = 3              # S psum banks per exp op
NBLK = NJ * HPC      # 32 S blocks per chunk
NGRP = (NBLK + GRP - 1) // GRP   # 11 exp groups per chunk
NPAIR = NJ // 2      # 8 j-tile pairs

LAST_RESULTS = None  # BassKernelResults of the most recent kernel() call


def _build_bass():
    import concourse.bass as bass
    import concourse.mybir as mybir
    import concourse.tile as tile
    from concourse import bacc

    f32 = mybir.dt.float32
    bf16 = mybir.dt.bfloat16
    i16 = mybir.dt.int16
    AF = mybir.ActivationFunctionType

    # Schraudolph exp: bf16 bit pattern of exp(x) ~ int16(x*128*log2(e) +
    # 127*128 - c).  Piecewise-linear 2^frac approx, max rel err ~3.4%;
    # softmax normalization cancels most of it (measured ~1% end-to-end
    # with 2 of 11 exp groups approximated).  Offloads ACT -> DVE.
    SCH_A = 128 * 1.4426950408889634
    SCH_B = 127.0 * 128 - 5.5
    N_SCH = int(os.environ.get("BASS_SCHRAUD", "0"))

    nc = bacc.Bacc("TRN2", target_bir_lowering=False, debug=False,
                   num_devices=NCORES)

    hT = nc.dram_tensor("hT", [IN_DIM, N], bf16, kind="ExternalInput").ap()
    Ab = nc.dram_tensor("Ab", [N, N], bf16, kind="ExternalInput").ap()
    wq = nc.dram_tensor("wq", [IN_DIM, 4 * D], bf16, kind="ExternalInput").ap()
    wk = nc.dram_tensor("wk", [IN_DIM, HPC * D], bf16, kind="ExternalInput").ap()
    wv = nc.dram_tensor("wv", [IN_DIM, HPC * D], bf16, kind="ExternalInput").ap()
    bq4 = nc.dram_tensor("bq4", [128, 1], f32, kind="ExternalInput").ap()
    bvb = nc.dram_tensor("bvb", [128, HPC * D], f32, kind="ExternalInput").ap()
    o = nc.dram_tensor("o", [128, N], f32, kind="ExternalOutput").ap()

    SC = 1.0 / math.sqrt(D)

    def bcast_free(ap_col, n):
        return bass.AP(tensor=ap_col.tensor, offset=ap_col.offset,
                       ap=[ap_col.ap[0], [0, n]])

    # block index for (head hh, j-tile t): strip r = 2*hh + t%2
    def blk(hh, t):
        return 4 * (t // 2) + 2 * hh + (t % 2)

    with (
        tile.TileContext(nc) as tc,
        tc.tile_pool(name="const", bufs=1) as cpool,
        tc.tile_pool(name="ps", bufs=2, space="PSUM") as pspool,
        tc.tile_pool(name="pod", bufs=2, space="PSUM") as podpool,
        tc.tile_pool(name="apool", bufs=2) as apool,
        tc.tile_pool(name="epool", bufs=2) as epool,
        tc.tile_pool(name="eapool", bufs=2) as eapool,
        tc.tile_pool(name="opool", bufs=2) as opool,
    ):
        # ---- constants / inputs into SBUF
        scratch = cpool.tile([128, CH], bf16, tag="scratch")
        nc.vector.memset(scratch, 0.0)
        ones_sb = cpool.tile([128, 1], bf16, tag="ones")
        nc.vector.memset(ones_sb, 1.0)

        # h^T in two half-tiles so K/Q projections can start on the first
        # half while the second is still in flight
        hT3 = hT.rearrange("(s p) n -> p s n", p=128)
        hT_half = [cpool.tile([128, 2, N // 2], bf16, tag=f"hT{c}",
                              name=f"hT{c}")
                   for c in range(2)]
        for q in range(2):
            nc.sync.dma_start(hT_half[0][:, :, q * CH:(q + 1) * CH],
                              hT3[:, :, q * CH:(q + 1) * CH])
        w_sb = {}
        for name, ap, m in (("q", wq, 4 * D), ("k", wk, HPC * D),
                            ("v", wv, HPC * D)):
            t = cpool.tile([128, 2, m], bf16, tag=f"w{name}")
            nc.sync.dma_start(t, ap.rearrange("(s p) m -> p s m", p=128))
            w_sb[name] = t
        bq4_sb = cpool.tile([128, 1], f32, tag="bq4")
        nc.sync.dma_start(bq4_sb, bq4)
        for q in range(2):
            nc.sync.dma_start(hT_half[1][:, :, q * CH:(q + 1) * CH],
                              hT3[:, :, N // 2 + q * CH:N // 2 + (q + 1) * CH])
        bvb_sb = cpool.tile([128, HPC * D], f32, tag="bvb")
        nc.sync.dma_start(bvb_sb, bvb)

        def hT_sb_cols(s, lo, hi):       # [lo, hi) within one half
            c = lo // (N // 2)
            assert (hi - 1) // (N // 2) == c
            return hT_half[c][:, s, lo - c * N // 2:hi - c * N // 2]

        qt4 = cpool.tile([128, N], bf16, tag="qt4")      # strips [h0|h0|h1|h1]
        kt4 = cpool.tile([128, NJ // 2 * 128], bf16, tag="kt4")
        Vt = cpool.tile([128, NJ * HPC * D], bf16, tag="vt")   # col = t*64+d

        A3 = Ab.rearrange("(t p) i -> p t i", p=128)
        a_tiles = [None] * NCH

        def emit_a_dma(ch):
            a_t = apool.tile([128, NJ, CH], bf16, tag="a")
            nc.sync.dma_start(a_t, A3[:, :, ch * CH:(ch + 1) * CH])
            a_tiles[ch] = a_t

        emit_a_dma(0)

        # ---- PE warm-up: a few dummy matmuls while input DMAs are in
        #      flight (kept small: a dense burst across all 8 cores trips
        #      the chip-wide power throttle and downclocks everything 1.2x)
        for _ in range(3):
            jt = podpool.tile([128, CH], f32, tag="od", name="junk")
            nc.tensor.matmul(jt, lhsT=scratch[:, 0:128], rhs=scratch,
                             start=True, stop=True)

        # ---- V projection helpers (all 16 tiles run inline in chunk 0)
        def emit_vproj_mms(vps, base, t0, cnt):
            for t in range(t0, t0 + cnt):
                for s in range(2):
                    nc.tensor.matmul(
                        vps[:, (t - base) * HPC * D:(t - base + 1) * HPC * D],
                        lhsT=hT_sb_cols(s, t * 128, (t + 1) * 128),
                        rhs=w_sb["v"][:, s, :],
                        start=(s == 0), stop=(s == 1),
                    )

        def emit_vproj_add(vps, t0):
            base = Vt[:, t0 * HPC * D:(t0 + 8) * HPC * D]
            out_ap = bass.AP(tensor=base.tensor, offset=base.offset,
                             ap=[base.ap[0], [HPC * D, 8], [1, HPC * D]])
            in_ap = bass.AP(tensor=vps.tensor, offset=vps.offset,
                            ap=[vps.ap[0], [HPC * D, 8], [1, HPC * D]])
            b_ap = bass.AP(tensor=bvb_sb.tensor, offset=bvb_sb.offset,
                           ap=[bvb_sb.ap[0], [0, 8], [1, HPC * D]])
            nc.vector.tensor_add(out_ap, in_ap, b_ap)

        # ---- K projection into packed strip layout.
        #      strip r holds K^T of head r//2 for tiles t = 2*q2 + r%2.
        #      c=0 (j-tiles 0-7 -> S groups 0-4) runs on the first hT half
        #      before the loop; c=1 is emitted inside chunk 0, group 0.
        def emit_kproj(c):
            kps = pspool.tile([128, CH], f32, tag="ps", name=f"kps{c}")
            for r in range(4):
                for s in range(2):
                    base = hT_half[c][:, s, (r % 2) * 128:(r % 2) * 128 + 128]
                    rhs = bass.AP(tensor=base.tensor, offset=base.offset,
                                  ap=[base.ap[0], [256, 4], [1, 128]])
                    nc.tensor.matmul(
                        kps[32 * r:32 * r + 32, :],
                        lhsT=w_sb["k"][:, s, (r // 2) * D:(r // 2 + 1) * D],
                        rhs=rhs,
                        start=(s == 0), stop=(s == 1),
                        tile_position=(0, 32 * r),
                    )
            return kps

        # bk dropped (cancels in softmax); cast on the idle ACT engine
        kps0 = emit_kproj(0)
        nc.scalar.activation(kt4[:, 0:CH], kps0[:, 0:CH], AF.Copy)

        # ---- Q projection (+bias, scaled 1/sqrt(D)), strip-replicated
        #      directly: wq columns are [h0|h0|h1|h1] (host-packed).
        def emit_qproj(quarter, on_act=False):
            sl = slice(quarter * CH, (quarter + 1) * CH)
            qps = pspool.tile([128, CH], f32, tag="ps", name="qps")
            for s in range(2):
                nc.tensor.matmul(qps, lhsT=w_sb["q"][:, s, :],
                                 rhs=hT_sb_cols(s, quarter * CH,
                                                (quarter + 1) * CH),
                                 start=(s == 0), stop=(s == 1))
            if on_act:
                nc.scalar.activation(qt4[:, sl], qps, AF.Identity,
                                     bias=bq4_sb, scale=SC)
            else:
                nc.vector.scalar_tensor_tensor(
                    qt4[:, sl], qps, SC, bcast_free(bq4_sb, CH),
                    op0=mybir.AluOpType.mult, op1=mybir.AluOpType.add,
                )

        emit_qproj(0, on_act=True)

        # ---- main loop helpers
        def emit_amult(e_t, ea_t, a_t, p, hh):
            if hh is None:   # both heads of pair p in one op (4D AP)
                b0 = 4 * p * CH
                eb = e_t[:, b0:b0 + CH]
                e_ap = bass.AP(tensor=eb.tensor, offset=eb.offset,
                               ap=[eb.ap[0], [2 * CH, 2], [CH, 2], [1, CH]])
                eab = ea_t[:, b0:b0 + CH]
                ea_ap = bass.AP(tensor=eab.tensor, offset=eab.offset,
                                ap=[eab.ap[0], [2 * CH, 2], [CH, 2], [1, CH]])
                ab = a_t[:, 2 * p, :]
                a_ap = bass.AP(tensor=ab.tensor, offset=ab.offset,
                               ap=[ab.ap[0], [0, 2], [CH, 2], [1, CH]])
                nc.vector.tensor_mul(ea_ap, e_ap, a_ap)
                return
            # blocks 4p+2hh, 4p+2hh+1 = head hh, j-tiles 2p, 2p+1: contiguous
            b0 = (4 * p + 2 * hh) * CH
            eb = e_t[:, b0:b0 + CH]
            e_ap = bass.AP(tensor=eb.tensor, offset=eb.offset,
                           ap=[eb.ap[0], [CH, 2], [1, CH]])
            eab = ea_t[:, b0:b0 + CH]
            ea_ap = bass.AP(tensor=eab.tensor, offset=eab.offset,
                            ap=[eab.ap[0], [CH, 2], [1, CH]])
            nc.vector.tensor_mul(ea_ap, e_ap, a_t[:, 2 * p:2 * p + 2, :])

        def emit_ph2_quad(od, e_t, ea_t, p, hh):
            for tp in range(2):
                t = 2 * p + tp
                first, last = (t == 0), (t == NJ - 1)
                bsl = slice(blk(hh, t) * CH, (blk(hh, t) + 1) * CH)
                nc.tensor.matmul(
                    od[32 * hh:32 * hh + 32, :],
                    lhsT=Vt[:, t * 64 + 32 * hh:t * 64 + 32 * hh + 32],
                    rhs=ea_t[:, bsl],
                    start=first, stop=last, tile_position=(0, 32 * hh),
                )
                nc.tensor.matmul(
                    od[64 + 32 * hh:65 + 32 * hh, :],
                    lhsT=ones_sb, rhs=e_t[:, bsl],
                    start=first, stop=last,
                    tile_position=(0, 64 + 32 * hh),
                )

        def emit_out(od, ch):
            o_sb = opool.tile([128, CH], f32, tag="o")
            nc.vector.tensor_copy(o_sb, od)
            nc.sync.dma_start(o[:, ch * CH:(ch + 1) * CH], o_sb)

        carry = None   # (od, e_t, ea_t, ch, quads) spill of previous chunk
        for ch in range(NCH):
            if ch + 1 < NCH:
                emit_a_dma(ch + 1)
            a_t = a_tiles[ch]
            e_t = epool.tile([128, NBLK * CH], bf16, tag="e")
            ea_t = eapool.tile([128, NBLK * CH], bf16, tag="ea")
            od = None

            # insert schedule.  A-mult TTs are merged per pair (ready after
            # the exp group covering block 4p+3) -> TT groups {1,2,3,5,6,7,
            # 9,10}; groups 4 and 8 stay TT-free for the Schraudolph op and
            # the q-projection stt.  Phase2 quads follow with a lag, h0 one
            # group before h1, spilling into the next chunk's groups 0-2.
            last = ch == NCH - 1
            amult_at = {}
            ph2_at = {}
            spill = []

            def sched_quad(gq, p, hh):
                if gq < NGRP:
                    ph2_at.setdefault(gq, []).append((p, hh))
                else:
                    spill.append((p, hh))

            for p in range(NPAIR):
                if last and p >= 6:      # tail: per-head TTs, minimal lag
                    for hh in range(HPC):
                        rg = (4 * p + 2 * hh + 1) // GRP
                        amult_at.setdefault(rg, []).append((p, hh))
                        sched_quad(min(rg + 1, NGRP - 1), p, hh)
                else:
                    rg = (4 * p + 3) // GRP
                    amult_at.setdefault(rg, []).append((p, None))
                    l0 = 3 if ch == 0 else 2
                    for hh in range(HPC):
                        sched_quad(rg + l0 + hh, p, hh)

            sch_groups = (4, 8)[:N_SCH]

            for g in range(NGRP):
                b_lo = g * GRP
                b_hi = min(b_lo + GRP, NBLK)
                ps = pspool.tile([128, GRP * CH], f32, tag="ps", name="sps")
                for b in range(b_lo, b_hi):
                    q2, r = b // 4, b % 4
                    nc.tensor.matmul(
                        ps[:, (b - b_lo) * CH:(b - b_lo + 1) * CH],
                        lhsT=kt4[32 * r:32 * r + 32, q2 * 128:(q2 + 1) * 128],
                        rhs=qt4[32 * r:32 * r + 32, ch * CH:(ch + 1) * CH],
                        start=True, stop=True,
                        tile_position=(32 * r, 0),
                    )
                if g in sch_groups:
                    nc.vector.tensor_scalar(
                        e_t[:, b_lo * CH:b_hi * CH].bitcast(i16),
                        ps[:, :(b_hi - b_lo) * CH], SCH_A, SCH_B,
                        op0=mybir.AluOpType.mult, op1=mybir.AluOpType.add)
                else:
                    nc.scalar.activation(
                        e_t[:, b_lo * CH:b_hi * CH],
                        ps[:, :(b_hi - b_lo) * CH], AF.Exp)

                for p, hh in amult_at.get(g, ()):
                    emit_amult(e_t, ea_t, a_t, p, hh)

                # drain previous chunk's spilled phase2 quads: 2 at group 0
                # (quad-free there), then 1 per group to avoid PE bursts
                if carry is not None and g <= 4:
                    cod, ce, cea, cch, cquads = carry
                    take = cquads[:2 if g == 0 else 1] if g < 4 else cquads
                    for p, hh in take:
                        emit_ph2_quad(cod, ce, cea, p, hh)
                    cquads = cquads[len(take):]
                    if not cquads:
                        emit_out(cod, cch)
                        carry = None
                    else:
                        carry = (cod, ce, cea, cch, cquads)

                if ch == 0:
                    if g == 0:
                        # K projection second half + cast (DVE: ACT queue
                        # must stay clear for the exp stream)
                        kps1 = emit_kproj(1)
                        nc.vector.tensor_copy(kt4[:, CH:2 * CH], kps1)
                    if g < 4:
                        if g == 0:
                            vps0 = podpool.tile([128, CH], f32, tag="od",
                                                name="vps0")
                        emit_vproj_mms(vps0, 0, 2 * g, 2)
                        if g == 3:
                            emit_vproj_add(vps0, 0)
                    elif g < 8:
                        if g == 4:
                            vps1 = podpool.tile([128, CH], f32, tag="od",
                                                name="vps1")
                        emit_vproj_mms(vps1, 8, 2 * g, 2)
                        if g == 7:
                            emit_vproj_add(vps1, 8)
                    elif g == 8:
                        emit_qproj(1)
                elif ch in (1, 2) and g == 6:
                    emit_qproj(ch + 1)

                for p, hh in ph2_at.get(g, ()):
                    if od is None:
                        od = podpool.tile([128, CH], f32, tag="od")
                    emit_ph2_quad(od, e_t, ea_t, p, hh)

            carry = (od, e_t, ea_t, ch, spill)

        cod, ce, cea, cch, cquads = carry
        for p, hh in cquads:
            emit_ph2_quad(cod, ce, cea, p, hh)
        emit_out(cod, cch)

    nc.finalize()
    return nc


def kernel(h, A, Wq, bq, Wk, bk, Wv, bv):
    global LAST_RESULTS
    from concourse.bass_utils import run_bass_kernel_spmd

    h = np.asarray(h, np.float32)
    A = np.asarray(A, np.float32)
    Wq = np.asarray(Wq, np.float32)
    Wk = np.asarray(Wk, np.float32)
    Wv = np.asarray(Wv, np.float32)
    bq = np.asarray(bq, np.float32)
    bv = np.asarray(bv, np.float32)

    hT = np.ascontiguousarray(h.transpose(0, 2, 1)).astype(ml_dtypes.bfloat16)
    Ab = np.ascontiguousarray(A.astype(ml_dtypes.bfloat16))  # [B, N, N]
    sc = np.float32(1.0 / math.sqrt(D))

    in_maps = []
    for c in range(NCORES):
        b = c // CORES_PER_B
        h0 = HPC * (c % CORES_PER_B)
        sl = slice(h0 * D, (h0 + HPC) * D)
        wq_h = [Wq[:, (h0 + k) * D:(h0 + k + 1) * D] for k in range(HPC)]
        wq_rep = np.concatenate([wq_h[0], wq_h[0], wq_h[1], wq_h[1]], axis=1)
        bq_h = [bq[(h0 + k) * D:(h0 + k + 1) * D] for k in range(HPC)]
        bq4 = np.concatenate([bq_h[0], bq_h[0], bq_h[1], bq_h[1]]) * sc
        in_maps.append({
            "hT": hT[b],
            "Ab": Ab[b],
            "wq": np.ascontiguousarray(wq_rep).astype(ml_dtypes.bfloat16),
            "wk": np.ascontiguousarray(Wk[:, sl]).astype(ml_dtypes.bfloat16),
            "wv": np.ascontiguousarray(Wv[:, sl]).astype(ml_dtypes.bfloat16),
            "bq4": np.ascontiguousarray(bq4.reshape(128, 1)),
            "bvb": np.ascontiguousarray(np.tile(bv[sl][None, :], (128, 1))),
        })

    nc = _build_bass()
    res = run_bass_kernel_spmd(
        nc, in_maps, core_ids=list(range(NCORES)),
        trace=os.environ.get("BASS_TRACE", "0") == "1",
    )
    LAST_RESULTS = res

    out = np.empty((B, HEADS, N, D), np.float32)
    for c in range(NCORES):
        b = c // CORES_PER_B
        h0 = HPC * (c % CORES_PER_B)
        oo = res.results[c]["o"]                  # [128, N] f32
        for hh in range(HPC):
            num = oo[hh * D:(hh + 1) * D, :]      # [32, N] unnormalized out^T
            den = oo[64 + 32 * hh, :]             # [N]
            out[b, h0 + hh] = (num / den[None, :]).T
    return out


# revision 54
# speedup vs baseline: 1.0133x; 1.0133x over previous
"""Adjacency-aware multi-head attention on 8 trn2 NeuronCores.

Math (per b, head k):
  Q = h[b] @ Wq[:, k] + bq[k]           [N, D]
  S[i, j] = (Q_i . K_j) / sqrt(D)
  P[j, i] = exp(S[i, j]) / sum_j exp(S[i, j])      (softmax over keys j)
  out[i, d] = sum_j P[j, i] * A[b, j, i] * V[j, d]

The K bias cancels: it adds g[i] = Q_i . bk to every score of query i,
and softmax over j is invariant to per-i shifts -> bk is dropped.

Sharding: 16 (b, head) pairs over 8 cores, 2 heads of the SAME b per core so
the A[b] stream is shared by both heads.

Device dataflow ([j, i] layout so A needs no transpose).  The steady state is
paced by the ACT engine (exp of all scores); everything else hides under it:
  - PE warm-up: dummy matmuls + first half of the V projection run during the
    input-DMA wait so the HAM clock gate reaches 2.4 GHz before the main loop.
  - Strip mapping r = 2*head + (t%2): j-tile t of head hh computes on PE row
    strip r.  Q^T is written strip-replicated directly by the Q projection
    (host passes Wq with columns [h0|h0|h1|h1]), no SBUF->SBUF copies.
  - K^T packed per strip, K projection runs as 8 matmuls of N=1024.
  - exp on ACT: PSUM -> SBUF bf16, 3 PSUM banks per op.
  - EA = E * A on DVE: one 4D-AP tensor_tensor per j-tile pair (2x bf16 mode).
  - phase 2 (eager, lagged ~2 exp-groups): per j-tile 4 streams into one PSUM
    tile via column tiling: out_h0 (cols 0-31), out_h1 (32-63), denom_h0
    (col 64), denom_h1 (col 96).
Device returns [128, N]: rows 0-31 outT_h0, 32-63 outT_h1, rows 64/96 the
softmax denominators.  Host does out = (outT / denom)^T plus the gather.
"""

import math
import os

import numpy as np
import ml_dtypes

B, N, IN_DIM = 2, 2048, 256
HEADS, D = 8, 32
NCORES = 8
HPC = 2              # heads per core
NJ = N // 128        # 16 j-tiles
NCH = 4              # i-chunks
CH = N // NCH        # 512
CORES_PER_B = NCORES // B
GRP = 3              # S psum banks per exp op
NBLK = NJ * HPC      # 32 S blocks per chunk
NGRP = (NBLK + GRP - 1) // GRP   # 11 exp groups per chunk
NPAIR = NJ // 2      # 8 j-tile pairs

LAST_RESULTS = None  # BassKernelResults of the most recent kernel() call


def _build_bass():
    import concourse.bass as bass
    import concourse.mybir as mybir
    import concourse.tile as tile
    from concourse import bacc

    f32 = mybir.dt.float32
    bf16 = mybir.dt.bfloat16
    i16 = mybir.dt.int16
    AF = mybir.ActivationFunctionType

    # Schraudolph exp: bf16 bit pattern of exp(x) ~ int16(x*128*log2(e) +
    # 127*128 - c).  Piecewise-linear 2^frac approx, max rel err ~3.4%;
    # softmax normalization cancels most of it (measured ~1% end-to-end
    # with 2 of 11 exp groups approximated).  Offloads ACT -> DVE.
    SCH_A = 128 * 1.4426950408889634
    SCH_B = 127.0 * 128 - 5.5
    N_SCH = int(os.environ.get("BASS_SCHRAUD", "0"))

    nc = bacc.Bacc("TRN2", target_bir_lowering=False, debug=False,
                   num_devices=NCORES)

    hT = nc.dram_tensor("hT", [IN_DIM, N], bf16, kind="ExternalInput").ap()
    Ab = nc.dram_tensor("Ab", [N, N], bf16, kind="ExternalInput").ap()
    wq = nc.dram_tensor("wq", [IN_DIM, 4 * D], bf16, kind="ExternalInput").ap()
    wk = nc.dram_tensor("wk", [IN_DIM, HPC * D], bf16, kind="ExternalInput").ap()
    wv = nc.dram_tensor("wv", [IN_DIM, HPC * D], bf16, kind="ExternalInput").ap()
    bq4 = nc.dram_tensor("bq4", [128, 1], f32, kind="ExternalInput").ap()
    bvb = nc.dram_tensor("bvb", [128, HPC * D], f32, kind="ExternalInput").ap()
    o = nc.dram_tensor("o", [128, N], f32, kind="ExternalOutput").ap()

    SC = 1.0 / math.sqrt(D)

    def bcast_free(ap_col, n):
        return bass.AP(tensor=ap_col.tensor, offset=ap_col.offset,
                       ap=[ap_col.ap[0], [0, n]])

    # block index for (head hh, j-tile t): strip r = 2*hh + t%2
    def blk(hh, t):
        return 4 * (t // 2) + 2 * hh + (t % 2)

    with (
        tile.TileContext(nc) as tc,
        tc.tile_pool(name="const", bufs=1) as cpool,
        tc.tile_pool(name="ps", bufs=2, space="PSUM") as pspool,
        tc.tile_pool(name="pod", bufs=2, space="PSUM") as podpool,
        tc.tile_pool(name="apool", bufs=2) as apool,
        tc.tile_pool(name="epool", bufs=2) as epool,
        tc.tile_pool(name="eapool", bufs=2) as eapool,
        tc.tile_pool(name="opool", bufs=2) as opool,
    ):
        # ---- constants / inputs into SBUF
        scratch = cpool.tile([128, CH], bf16, tag="scratch")
        nc.vector.memset(scratch, 0.0)
        ones_sb = cpool.tile([128, 1], bf16, tag="ones")
        nc.vector.memset(ones_sb, 1.0)

        # h^T in two half-tiles so K/Q projections can start on the first
        # half while the second is still in flight
        hT3 = hT.rearrange("(s p) n -> p s n", p=128)
        hT_half = [cpool.tile([128, 2, N // 2], bf16, tag=f"hT{c}",
                              name=f"hT{c}")
                   for c in range(2)]
        for q in range(2):
            nc.sync.dma_start(hT_half[0][:, :, q * CH:(q + 1) * CH],
                              hT3[:, :, q * CH:(q + 1) * CH])
        w_sb = {}
        for name, ap, m in (("q", wq, 4 * D), ("k", wk, HPC * D),
                            ("v", wv, HPC * D)):
            t = cpool.tile([128, 2, m], bf16, tag=f"w{name}")
            nc.sync.dma_start(t, ap.rearrange("(s p) m -> p s m", p=128))
            w_sb[name] = t
        bq4_sb = cpool.tile([128, 1], f32, tag="bq4")
        nc.sync.dma_start(bq4_sb, bq4)
        for q in range(2):
            nc.sync.dma_start(hT_half[1][:, :, q * CH:(q + 1) * CH],
                              hT3[:, :, N // 2 + q * CH:N // 2 + (q + 1) * CH])
        bvb_sb = cpool.tile([128, HPC * D], f32, tag="bvb")
        nc.sync.dma_start(bvb_sb, bvb)

        def hT_sb_cols(s, lo, hi):       # [lo, hi) within one half
            c = lo // (N // 2)
            assert (hi - 1) // (N // 2) == c
            return hT_half[c][:, s, lo - c * N // 2:hi - c * N // 2]

        qt4 = cpool.tile([128, N], bf16, tag="qt4")      # strips [h0|h0|h1|h1]
        kt4 = cpool.tile([128, NJ // 2 * 128], bf16, tag="kt4")
        Vt = cpool.tile([128, NJ * HPC * D], bf16, tag="vt")   # col = t*64+d

        A3 = Ab.rearrange("(t p) i -> p t i", p=128)
        a_tiles = [None] * NCH

        def emit_a_dma(ch):
            a_t = apool.tile([128, NJ, CH], bf16, tag="a")
            nc.sync.dma_start(a_t, A3[:, :, ch * CH:(ch + 1) * CH])
            a_tiles[ch] = a_t

        emit_a_dma(0)

        # ---- PE warm-up: a few dummy matmuls while input DMAs are in
        #      flight (kept small: a dense burst across all 8 cores trips
        #      the chip-wide power throttle and downclocks everything 1.2x)
        for _ in range(3):
            jt = podpool.tile([128, CH], f32, tag="od", name="junk")
            nc.tensor.matmul(jt, lhsT=scratch[:, 0:128], rhs=scratch,
                             start=True, stop=True)

        # ---- V projection helpers (all 16 tiles run inline in chunk 0)
        def emit_vproj_mms(vps, base, t0, cnt):
            for t in range(t0, t0 + cnt):
                for s in range(2):
                    nc.tensor.matmul(
                        vps[:, (t - base) * HPC * D:(t - base + 1) * HPC * D],
                        lhsT=hT_sb_cols(s, t * 128, (t + 1) * 128),
                        rhs=w_sb["v"][:, s, :],
                        start=(s == 0), stop=(s == 1),
                    )

        def emit_vproj_add(vps, t0):
            base = Vt[:, t0 * HPC * D:(t0 + 8) * HPC * D]
            out_ap = bass.AP(tensor=base.tensor, offset=base.offset,
                             ap=[base.ap[0], [HPC * D, 8], [1, HPC * D]])
            in_ap = bass.AP(tensor=vps.tensor, offset=vps.offset,
                            ap=[vps.ap[0], [HPC * D, 8], [1, HPC * D]])
            b_ap = bass.AP(tensor=bvb_sb.tensor, offset=bvb_sb.offset,
                           ap=[bvb_sb.ap[0], [0, 8], [1, HPC * D]])
            nc.vector.tensor_add(out_ap, in_ap, b_ap)

        # ---- K projection into packed strip layout.
        #      strip r holds K^T of head r//2 for tiles t = 2*q2 + r%2.
        #      c=0 (j-tiles 0-7 -> S groups 0-4) runs on the first hT half
        #      before the loop; c=1 is emitted inside chunk 0, group 0.
        def emit_kproj(c):
            kps = pspool.tile([128, CH], f32, tag="ps", name=f"kps{c}")
            for r in range(4):
                for s in range(2):
                    base = hT_half[c][:, s, (r % 2) * 128:(r % 2) * 128 + 128]
                    rhs = bass.AP(tensor=base.tensor, offset=base.offset,
                                  ap=[base.ap[0], [256, 4], [1, 128]])
                    nc.tensor.matmul(
                        kps[32 * r:32 * r + 32, :],
                        lhsT=w_sb["k"][:, s, (r // 2) * D:(r // 2 + 1) * D],
                        rhs=rhs,
                        start=(s == 0), stop=(s == 1),
                        tile_position=(0, 32 * r),
                    )
            return kps

        # bk dropped (cancels in softmax); cast on the idle ACT engine
        kps0 = emit_kproj(0)
        nc.scalar.activation(kt4[:, 0:CH], kps0[:, 0:CH], AF.Copy)

        # ---- Q projection (+bias, scaled 1/sqrt(D)), strip-replicated
        #      directly: wq columns are [h0|h0|h1|h1] (host-packed).
        def emit_qproj(quarter, on_act=False):
            sl = slice(quarter * CH, (quarter + 1) * CH)
            qps = pspool.tile([128, CH], f32, tag="ps", name="qps")
            for s in range(2):
                nc.tensor.matmul(qps, lhsT=w_sb["q"][:, s, :],
                                 rhs=hT_sb_cols(s, quarter * CH,
                                                (quarter + 1) * CH),
                                 start=(s == 0), stop=(s == 1))
            if on_act:
                nc.scalar.activation(qt4[:, sl], qps, AF.Identity,
                                     bias=bq4_sb, scale=SC)
            else:
                nc.vector.scalar_tensor_tensor(
                    qt4[:, sl], qps, SC, bcast_free(bq4_sb, CH),
                    op0=mybir.AluOpType.mult, op1=mybir.AluOpType.add,
                )

        emit_qproj(0, on_act=True)

        # ---- main loop helpers
        def emit_amult(e_t, ea_t, a_t, p, hh):
            if hh is None:   # both heads of pair p in one op (4D AP)
                b0 = 4 * p * CH
                eb = e_t[:, b0:b0 + CH]
                e_ap = bass.AP(tensor=eb.tensor, offset=eb.offset,
                               ap=[eb.ap[0], [2 * CH, 2], [CH, 2], [1, CH]])
                eab = ea_t[:, b0:b0 + CH]
                ea_ap = bass.AP(tensor=eab.tensor, offset=eab.offset,
                                ap=[eab.ap[0], [2 * CH, 2], [CH, 2], [1, CH]])
                ab = a_t[:, 2 * p, :]
                a_ap = bass.AP(tensor=ab.tensor, offset=ab.offset,
                               ap=[ab.ap[0], [0, 2], [CH, 2], [1, CH]])
                nc.vector.tensor_mul(ea_ap, e_ap, a_ap)
                return
            # blocks 4p+2hh, 4p+2hh+1 = head hh, j-tiles 2p, 2p+1: contiguous
            b0 = (4 * p + 2 * hh) * CH
            eb = e_t[:, b0:b0 + CH]
            e_ap = bass.AP(tensor=eb.tensor, offset=eb.offset,
                           ap=[eb.ap[0], [CH, 2], [1, CH]])
            eab = ea_t[:, b0:b0 + CH]
            ea_ap = bass.AP(tensor=eab.tensor, offset=eab.offset,
                            ap=[eab.ap[0], [CH, 2], [1, CH]])
            nc.vector.tensor_mul(ea_ap, e_ap, a_t[:, 2 * p:2 * p + 2, :])

        def emit_ph2_quad(od, e_t, ea_t, p, hh):
            for tp in range(2):
                t = 2 * p + tp
                first, last = (t == 0), (t == NJ - 1)
                bsl = slice(blk(hh, t) * CH, (blk(hh, t) + 1) * CH)
                nc.tensor.matmul(
                    od[32 * hh:32 * hh + 32, :],
                    lhsT=Vt[:, t * 64 + 32 * hh:t * 64 + 32 * hh + 32],
                    rhs=ea_t[:, bsl],
                    start=first, stop=last, tile_position=(0, 32 * hh),
                )
                nc.tensor.matmul(
                    od[64 + 32 * hh:65 + 32 * hh, :],
                    lhsT=ones_sb, rhs=e_t[:, bsl],
                    start=first, stop=last,
                    tile_position=(0, 64 + 32 * hh),
                )

        def emit_out(od, ch):
            o_sb = opool.tile([128, CH], f32, tag="o")
            nc.vector.tensor_copy(o_sb, od)
            nc.sync.dma_start(o[:, ch * CH:(ch + 1) * CH], o_sb)

        carry = None   # (od, e_t, ea_t, ch, quads) spill of previous chunk
        for ch in range(NCH):
            if ch + 1 < NCH:
                emit_a_dma(ch + 1)
            a_t = a_tiles[ch]
            e_t = epool.tile([128, NBLK * CH], bf16, tag="e")
            ea_t = eapool.tile([128, NBLK * CH], bf16, tag="ea")
            od = None

            # insert schedule.  A-mult TTs are merged per pair (ready after
            # the exp group covering block 4p+3) -> TT groups {1,2,3,5,6,7,
            # 9,10}; groups 4 and 8 stay TT-free for the Schraudolph op and
            # the q-projection stt.  Phase2 quads follow with a lag, h0 one
            # group before h1, spilling into the next chunk's groups 0-2.
            last = ch == NCH - 1
            amult_at = {}
            ph2_at = {}
            spill = []

            def sched_quad(gq, p, hh):
                if gq < NGRP:
                    ph2_at.setdefault(gq, []).append((p, hh))
                else:
                    spill.append((p, hh))

            for p in range(NPAIR):
                if last and p >= 6:      # tail: per-head TTs, minimal lag
                    for hh in range(HPC):
                        rg = (4 * p + 2 * hh + 1) // GRP
                        amult_at.setdefault(rg, []).append((p, hh))
                        sched_quad(min(rg + 1, NGRP - 1), p, hh)
                else:
                    rg = (4 * p + 3) // GRP
                    amult_at.setdefault(rg, []).append((p, None))
                    l0 = 3 if ch == 0 else 2
                    for hh in range(HPC):
                        sched_quad(rg + l0 + hh, p, hh)

            sch_groups = (4, 8)[:N_SCH]

            for g in range(NGRP):
                b_lo = g * GRP
                b_hi = min(b_lo + GRP, NBLK)
                ps = pspool.tile([128, GRP * CH], f32, tag="ps", name="sps")
                for b in range(b_lo, b_hi):
                    q2, r = b // 4, b % 4
                    nc.tensor.matmul(
                        ps[:, (b - b_lo) * CH:(b - b_lo + 1) * CH],
                        lhsT=kt4[32 * r:32 * r + 32, q2 * 128:(q2 + 1) * 128],
                        rhs=qt4[32 * r:32 * r + 32, ch * CH:(ch + 1) * CH],
                        start=True, stop=True,
                        tile_position=(32 * r, 0),
                    )
                if g in sch_groups:
                    nc.vector.tensor_scalar(
                        e_t[:, b_lo * CH:b_hi * CH].bitcast(i16),
                        ps[:, :(b_hi - b_lo) * CH], SCH_A, SCH_B,
                        op0=mybir.AluOpType.mult, op1=mybir.AluOpType.add)
                else:
                    nc.scalar.activation(
                        e_t[:, b_lo * CH:b_hi * CH],
                        ps[:, :(b_hi - b_lo) * CH], AF.Exp)

                for p, hh in amult_at.get(g, ()):
                    emit_amult(e_t, ea_t, a_t, p, hh)

                # drain previous chunk's spilled phase2 quads: 2 at group 0
                # (quad-free there), then 1 per group to avoid PE bursts
                if carry is not None and g <= 4:
                    cod, ce, cea, cch, cquads = carry
                    take = cquads[:2 if g == 0 else 1] if g < 4 else cquads
                    for p, hh in take:
                        emit_ph2_quad(cod, ce, cea, p, hh)
                    cquads = cquads[len(take):]
                    if not cquads:
                        emit_out(cod, cch)
                        carry = None
                    else:
                        carry = (cod, ce, cea, cch, cquads)

                if ch == 0:
                    if g == 0:
                        # K projection second half + cast (DVE: ACT queue
                        # must stay clear for the exp stream)
                        kps1 = emit_kproj(1)
                        nc.vector.tensor_copy(kt4[:, CH:2 * CH], kps1)
                    if g < 4:
                        if g == 0:
                            vps0 = podpool.tile([128, CH], f32, tag="od",
                                                name="vps0")
                        emit_vproj_mms(vps0, 0, 2 * g, 2)
                        if g == 3:
                            emit_vproj_add(vps0, 0)
                    elif g < 8:
                        if g == 4:
                            vps1 = podpool.tile([128, CH], f32, tag="od",
                                                name="vps1")
                        emit_vproj_mms(vps1, 8, 2 * g, 2)
                        if g == 7:
                            emit_vproj_add(vps1, 8)
                    elif g == 8:
                        emit_qproj(1)
                elif ch in (1, 2) and g == 8:
                    emit_qproj(ch + 1)

                for p, hh in ph2_at.get(g, ()):
                    if od is None:
                        od = podpool.tile([128, CH], f32, tag="od")
                    emit_ph2_quad(od, e_t, ea_t, p, hh)

            carry = (od, e_t, ea_t, ch, spill)

        cod, ce, cea, cch, cquads = carry
        for p, hh in cquads:
            emit_ph2_quad(cod, ce, cea, p, hh)
        emit_out(cod, cch)

    nc.finalize()
    return nc


def kernel(h, A, Wq, bq, Wk, bk, Wv, bv):
    global LAST_RESULTS
    from concourse.bass_utils import run_bass_kernel_spmd

    h = np.asarray(h, np.float32)
    A = np.asarray(A, np.float32)
    Wq = np.asarray(Wq, np.float32)
    Wk = np.asarray(Wk, np.float32)
    Wv = np.asarray(Wv, np.float32)
    bq = np.asarray(bq, np.float32)
    bv = np.asarray(bv, np.float32)

    hT = np.ascontiguousarray(h.transpose(0, 2, 1)).astype(ml_dtypes.bfloat16)
    Ab = np.ascontiguousarray(A.astype(ml_dtypes.bfloat16))  # [B, N, N]
    sc = np.float32(1.0 / math.sqrt(D))

    in_maps = []
    for c in range(NCORES):
        b = c // CORES_PER_B
        h0 = HPC * (c % CORES_PER_B)
        sl = slice(h0 * D, (h0 + HPC) * D)
        wq_h = [Wq[:, (h0 + k) * D:(h0 + k + 1) * D] for k in range(HPC)]
        wq_rep = np.concatenate([wq_h[0], wq_h[0], wq_h[1], wq_h[1]], axis=1)
        bq_h = [bq[(h0 + k) * D:(h0 + k + 1) * D] for k in range(HPC)]
        bq4 = np.concatenate([bq_h[0], bq_h[0], bq_h[1], bq_h[1]]) * sc
        in_maps.append({
            "hT": hT[b],
            "Ab": Ab[b],
            "wq": np.ascontiguousarray(wq_rep).astype(ml_dtypes.bfloat16),
            "wk": np.ascontiguousarray(Wk[:, sl]).astype(ml_dtypes.bfloat16),
            "wv": np.ascontiguousarray(Wv[:, sl]).astype(ml_dtypes.bfloat16),
            "bq4": np.ascontiguousarray(bq4.reshape(128, 1)),
            "bvb": np.ascontiguousarray(np.tile(bv[sl][None, :], (128, 1))),
        })

    nc = _build_bass()
    res = run_bass_kernel_spmd(
        nc, in_maps, core_ids=list(range(NCORES)),
        trace=os.environ.get("BASS_TRACE", "0") == "1",
    )
    LAST_RESULTS = res

    out = np.empty((B, HEADS, N, D), np.float32)
    for c in range(NCORES):
        b = c // CORES_PER_B
        h0 = HPC * (c % CORES_PER_B)
        oo = res.results[c]["o"]                  # [128, N] f32
        for hh in range(HPC):
            num = oo[hh * D:(hh + 1) * D, :]      # [32, N] unnormalized out^T
            den = oo[64 + 32 * hh, :]             # [N]
            out[b, h0 + hh] = (num / den[None, :]).T
    return out
